# revision 7
# baseline (speedup 1.0000x reference)
"""Trainium2 Bass kernel for nn_Block (pre-LN transformer block).

B=256, T=256, D=384, H=6, HS=64, FFN=1536. Data-parallel over batch:
32 batch elements per core x 8 cores, no collectives.

Per batch element (all matmuls float32r, PSUM f32 accumulate):
  LN1 (bn_stats/bn_aggr + sqrt + reciprocal + fused tensor_scalar)
  -> PE-transpose x_ln -> x_lnT [d,t]
  -> qT/kT (packed 2 heads per 128 partitions), v token-major
  -> scores t-major (K=64 matmul), exp via ACT (scale=1/sqrt(384),
     accum_out gives sumexp free), reciprocal, normalize,
     PE-transpose softmax -> wT [s,t]
  -> attT [e,t] = v.T @ wT   (d-on-partitions, ready for proj)
  -> proj + b_proj (ones-row K=1 matmul) + residual
  -> LN2 -> PE-transpose -> hT
  -> FFN1 (h1T = relu(w1.T @ hT + b1), bias per-partition via DVE)
  -> FFN2 + b2 (ones-row) + residual -> out
LN affine folding (host, exact): wq/wk/wv *= g1 rows; w1 *= g2 rows;
b1_eff = b1 + be2 @ w1. Requires be1 == 0 (true for this problem).
"""
import math

import numpy as np

import concourse.mybir as mybir
import concourse.tile as tile
from concourse import bacc
from concourse.bass_utils import run_bass_kernel_spmd
from concourse.masks import make_identity

P = 128
D = 384
T = 256
H = 6
HS = 64
F = 4 * D          # 1536
B_LOC = 32         # batch elements per core
N_CORES = 8
EPS = 1e-5
SCALE = 1.0 / math.sqrt(D)

_CACHE = {}


def _build():
    nc = bacc.Bacc("TRN2", target_bir_lowering=False)
    f32 = mybir.dt.float32
    f32r = mybir.dt.float32r

    x_d = nc.dram_tensor("x", [B_LOC, T, D], f32, kind="ExternalInput")
    wq_d = nc.dram_tensor("wqp", [D, D], f32r, kind="ExternalInput")
    wk_d = nc.dram_tensor("wkp", [D, D], f32r, kind="ExternalInput")
    wv_d = nc.dram_tensor("wvp", [D, D], f32r, kind="ExternalInput")
    wp_d = nc.dram_tensor("wpp", [D, D], f32r, kind="ExternalInput")
    w1_d = nc.dram_tensor("w1p", [D, F], f32r, kind="ExternalInput")
    w2_d = nc.dram_tensor("w2p", [F, D], f32r, kind="ExternalInput")
    bp_d = nc.dram_tensor("bpp", [1, D], f32r, kind="ExternalInput")
    b1_d = nc.dram_tensor("b1p", [P, F // P], f32, kind="ExternalInput")
    b2_d = nc.dram_tensor("b2p", [1, D], f32r, kind="ExternalInput")
    out_d = nc.dram_tensor("out", [B_LOC, T, D], f32, kind="ExternalOutput")

    with tile.TileContext(nc) as tc:
        with (
            tc.tile_pool(name="wts", bufs=1) as wts,
            tc.tile_pool(name="act", bufs=2) as act,
            tc.tile_pool(name="ps2", bufs=2, space="PSUM") as ps2,
            tc.tile_pool(name="ps3", bufs=2, space="PSUM") as ps3,
            tc.tile_pool(name="pst", bufs=2, space="PSUM") as pst,
        ):
            # ---- load weights once ----
            wq_sb = wts.tile([P, 3, D], f32r, name="wq_sb")
            nc.gpsimd.dma_start(wq_sb, wq_d.ap().rearrange("(c p) n -> p c n", p=P))
            wk_sb = wts.tile([P, 3, D], f32r, name="wk_sb")
            nc.gpsimd.dma_start(wk_sb, wk_d.ap().rearrange("(c p) n -> p c n", p=P))
            wv_sb = wts.tile([P, 3, D], f32r, name="wv_sb")
            nc.gpsimd.dma_start(wv_sb, wv_d.ap().rearrange("(c p) n -> p c n", p=P))
            wp_sb = wts.tile([HS, H, D], f32r, name="wp_sb")
            nc.gpsimd.dma_start(wp_sb, wp_d.ap().rearrange("(h e) n -> e h n", e=HS))
            w1_sb = wts.tile([P, 3, F], f32r, name="w1_sb")
            nc.gpsimd.dma_start(w1_sb, w1_d.ap().rearrange("(c p) n -> p c n", p=P))
            w2_sb = wts.tile([P, 12, D], f32r, name="w2_sb")
            nc.gpsimd.dma_start(w2_sb, w2_d.ap().rearrange("(c p) n -> p c n", p=P))
            bp_sb = wts.tile([1, D], f32r, name="bp_sb")
            nc.gpsimd.dma_start(bp_sb, bp_d.ap())
            b1_sb = wts.tile([P, F // P], f32, name="b1_sb")
            nc.gpsimd.dma_start(b1_sb, b1_d.ap())
            b2_sb = wts.tile([1, D], f32r, name="b2_sb")
            nc.gpsimd.dma_start(b2_sb, b2_d.ap())

            ident = wts.tile([P, P], f32, name="ident")
            make_identity(nc, ident)
            ones_f = wts.tile([1, P], f32, name="ones_f")
            nc.vector.memset(ones_f, 1.0)
            ones_r = wts.tile([1, P], f32r, name="ones_r")
            nc.vector.tensor_copy(ones_r, ones_f)
            eps_t = wts.tile([P, 1], f32, name="eps_t")
            nc.vector.memset(eps_t, EPS)

            def layernorm(dst, src):
                # dst[:, tc2, :] = LN(src[:, tc2, :]) for tc2 in 0..1  (no affine)
                for c2 in range(2):
                    stats = act.tile([P, 6], f32, tag="ln_stats", name="stats")
                    nc.vector.bn_stats(stats, src[:, c2, :])
                    mv = act.tile([P, 2], f32, tag="ln_mv", name="mv")
                    nc.vector.bn_aggr(mv, stats)
                    std = act.tile([P, 1], f32, tag="ln_std", name="std")
                    nc.scalar.activation(
                        std, mv[:, 1:2], mybir.ActivationFunctionType.Sqrt,
                        bias=eps_t, scale=1.0,
                    )
                    rstd = act.tile([P, 1], f32, tag="ln_rstd", name="rstd")
                    nc.vector.reciprocal(rstd, std)
                    nc.vector.tensor_scalar(
                        dst[:, c2, :], src[:, c2, :],
                        scalar1=mv[:, 0:1], scalar2=rstd,
                        op0=mybir.AluOpType.subtract, op1=mybir.AluOpType.mult,
                    )

            def transpose3(dst, src):
                # src [P, 2, 384] token-major -> dst [P, 3, 256] f32r (d-major)
                for dc in range(3):
                    tp = pst.tile([P, T], f32, tag="tp", name="tp")
                    for c2 in range(2):
                        nc.tensor.transpose(
                            tp[:, c2 * P:(c2 + 1) * P],
                            src[:, c2, dc * P:(dc + 1) * P], ident,
                        )
                    nc.vector.tensor_copy(dst[:, dc, :], tp)

            for b in range(B_LOC):
                x_sb = act.tile([P, 2, D], f32, tag="x", name="x_sb")
                nc.gpsimd.dma_start(
                    x_sb, x_d.ap()[b].rearrange("(c p) d -> p c d", p=P))

                xln = act.tile([P, 2, D], f32, tag="xln", name="xln")
                layernorm(xln, x_sb)
                xlnT = act.tile([P, 3, T], f32r, tag="xlnT", name="xlnT")
                transpose3(xlnT, xln)

                # qT / kT: 3 groups of 2 heads
                qT = act.tile([P, 3, T], f32r, tag="qT", name="qT")
                kT = act.tile([P, 3, T], f32r, tag="kT", name="kT")
                for g in range(3):
                    for dst, w in ((qT, wq_sb), (kT, wk_sb)):
                        mm = ps2.tile([P, T], f32, tag="mm256", name="mm")
                        for c in range(3):
                            nc.tensor.matmul(
                                mm, w[:, c, g * P:(g + 1) * P], xlnT[:, c, :],
                                start=(c == 0), stop=(c == 2),
                            )
                        nc.vector.tensor_copy(dst[:, g, :], mm)

                # v token-major [s, all-heads]
                v_sb = act.tile([P, 2, D], f32r, tag="v", name="v_sb")
                for sc in range(2):
                    vm = ps3.tile([P, D], f32, tag="mm384", name="vm")
                    for c in range(3):
                        nc.tensor.matmul(
                            vm, xlnT[:, c, sc * P:(sc + 1) * P], wv_sb[:, c, :],
                            start=(c == 0), stop=(c == 2),
                        )
                    nc.scalar.copy(v_sb[:, sc, :], vm)

                # attention per head
                attT = act.tile([HS, H, T], f32r, tag="attT", name="attT")
                for g in range(3):
                    for half in range(2):
                        h0 = half * HS
                        qh = qT[h0:h0 + HS, g, :]
                        kh = kT[h0:h0 + HS, g, :]
                        wexp = act.tile([P, 2, T], f32, tag="wexp", name="wexp")
                        sume = act.tile([P, 2], f32, tag="sume", name="sume")
                        rec = act.tile([P, 2], f32, tag="rec", name="rec")
                        wn = act.tile([P, 2, T], f32, tag="wn", name="wn")
                        for tc2 in range(2):
                            sc_ps = pst.tile([P, T], f32, tag="tp", name="sc_ps")
                            nc.tensor.matmul(
                                sc_ps, qh[:, tc2 * P:(tc2 + 1) * P], kh,
                                start=True, stop=True,
                            )
                            nc.scalar.activation(
                                wexp[:, tc2, :], sc_ps,
                                mybir.ActivationFunctionType.Exp,
                                scale=SCALE, accum_out=sume[:, tc2:tc2 + 1],
                            )
                            nc.vector.reciprocal(
                                rec[:, tc2:tc2 + 1], sume[:, tc2:tc2 + 1])
                            nc.vector.tensor_scalar_mul(
                                wn[:, tc2, :], in0=wexp[:, tc2, :],
                                scalar1=rec[:, tc2:tc2 + 1],
                            )
                        # transpose normalized softmax: wn [t, s] -> wT [s, t]
                        wT = act.tile([P, 2, T], f32r, tag="wT", name="wT")
                        for sc in range(2):
                            tp2 = pst.tile([P, T], f32, tag="tp", name="tp2")
                            for tc2 in range(2):
                                nc.tensor.transpose(
                                    tp2[:, tc2 * P:(tc2 + 1) * P],
                                    wn[:, tc2, sc * P:(sc + 1) * P], ident,
                                )
                            nc.scalar.copy(wT[:, sc, :], tp2)
                        h = g * 2 + half
                        ap_ps = ps2.tile([HS, T], f32, tag="ath", name="ap_ps")
                        for sc in range(2):
                            nc.tensor.matmul(
                                ap_ps,
                                v_sb[:, sc, h * HS:(h + 1) * HS],
                                wT[:, sc, :],
                                start=(sc == 0), stop=(sc == 1),
                            )
                        nc.vector.tensor_copy(attT[:, h, :], ap_ps)

                # proj + b_proj + residual -> x2
                x2 = act.tile([P, 2, D], f32, tag="x2", name="x2")
                for tc2 in range(2):
                    yp = ps3.tile([P, D], f32, tag="mm384", name="yp")
                    for h in range(H):
                        nc.tensor.matmul(
                            yp, attT[:, h, tc2 * P:(tc2 + 1) * P], wp_sb[:, h, :],
                            start=(h == 0), stop=False,
                        )
                    nc.tensor.matmul(yp, ones_r, bp_sb, start=False, stop=True)
                    nc.vector.tensor_tensor(
                        x2[:, tc2, :], yp, x_sb[:, tc2, :],
                        op=mybir.AluOpType.add,
                    )

                # LN2 -> hT
                hln = act.tile([P, 2, D], f32, tag="hln", name="hln")
                layernorm(hln, x2)
                hT = act.tile([P, 3, T], f32r, tag="hT", name="hT")
                transpose3(hT, hln)

                # FFN1: h1T[f-chunk] = relu(w1.T @ hT + b1)
                h1T = act.tile([P, 12, T], f32r, tag="h1T", name="h1T")
                for f in range(12):
                    fm = ps2.tile([P, T], f32, tag="mm256", name="fm")
                    for c in range(3):
                        nc.tensor.matmul(
                            fm, w1_sb[:, c, f * P:(f + 1) * P], hT[:, c, :],
                            start=(c == 0), stop=(c == 2),
                        )
                    nc.vector.tensor_scalar(
                        h1T[:, f, :], fm,
                        scalar1=b1_sb[:, f:f + 1], scalar2=0.0,
                        op0=mybir.AluOpType.add, op1=mybir.AluOpType.max,
                    )

                # FFN2 + b2 + residual -> out
                o_sb = act.tile([P, 2, D], f32, tag="o", name="o_sb")
                for tc2 in range(2):
                    op = ps3.tile([P, D], f32, tag="mm384", name="op")
                    for f in range(12):
                        nc.tensor.matmul(
                            op, h1T[:, f, tc2 * P:(tc2 + 1) * P], w2_sb[:, f, :],
                            start=(f == 0), stop=False,
                        )
                    nc.tensor.matmul(op, ones_r, b2_sb, start=False, stop=True)
                    nc.vector.tensor_tensor(
                        o_sb[:, tc2, :], op, x2[:, tc2, :],
                        op=mybir.AluOpType.add,
                    )
                nc.gpsimd.dma_start(
                    out_d.ap()[b].rearrange("(c p) d -> p c d", p=P), o_sb)

    nc.compile()
    return nc


def kernel(**inputs):
    x = np.ascontiguousarray(np.asarray(inputs["x"], dtype=np.float32))
    wq = np.asarray(inputs["wq"], dtype=np.float32)
    wk = np.asarray(inputs["wk"], dtype=np.float32)
    wv = np.asarray(inputs["wv"], dtype=np.float32)
    w_proj = np.asarray(inputs["w_proj"], dtype=np.float32)
    b_proj = np.asarray(inputs["b_proj"], dtype=np.float32)
    w1 = np.asarray(inputs["w1"], dtype=np.float32)
    b1 = np.asarray(inputs["b1"], dtype=np.float32)
    w2 = np.asarray(inputs["w2"], dtype=np.float32)
    b2 = np.asarray(inputs["b2"], dtype=np.float32)
    g1 = np.asarray(inputs["g1"], dtype=np.float32)
    be1 = np.asarray(inputs["be1"], dtype=np.float32)
    g2 = np.asarray(inputs["g2"], dtype=np.float32)
    be2 = np.asarray(inputs["be2"], dtype=np.float32)

    assert np.abs(be1).max() == 0.0, "be1 folding not implemented"

    # fold LN affines (exact): g into weight rows, be2 into b1
    wq_p = np.ascontiguousarray(
        (g1[:, None, None] * wq.transpose(1, 0, 2)).reshape(D, D))
    wk_p = np.ascontiguousarray(
        (g1[:, None, None] * wk.transpose(1, 0, 2)).reshape(D, D))
    wv_p = np.ascontiguousarray(
        (g1[:, None, None] * wv.transpose(1, 0, 2)).reshape(D, D))
    w1_p = np.ascontiguousarray(g2[:, None] * w1)
    b1_eff = b1 + be2 @ w1
    b1_p = np.ascontiguousarray(b1_eff.reshape(F // P, P).T)  # [P, 12]

    if "nc" not in _CACHE:
        _CACHE["nc"] = _build()
    nc = _CACHE["nc"]

    weights = {
        "wqp": wq_p, "wkp": wk_p, "wvp": wv_p,
        "wpp": np.ascontiguousarray(w_proj),
        "w1p": w1_p, "w2p": np.ascontiguousarray(w2),
        "bpp": b_proj.reshape(1, D), "b1p": b1_p, "b2p": b2.reshape(1, D),
    }
    in_maps = [
        {"x": x[c * B_LOC:(c + 1) * B_LOC], **weights} for c in range(N_CORES)
    ]
    last_exc = None
    for _attempt in range(3):
        try:
            res = run_bass_kernel_spmd(
                nc, in_maps, core_ids=list(range(N_CORES)))
            return np.concatenate([r["out"] for r in res.results], axis=0)
        except Exception as e:  # transient NRT_EXEC_UNIT_UNRECOVERABLE on cold start
            last_exc = e
    raise last_exc
